# revision 9
# baseline (speedup 1.0000x reference)
"""
Trainium2 Bass kernel for nn_ARqGPS (autoregressive qGPS log-amplitude).

Math (validated vs reference):
  eps_sel[b,m,t] = epsilon[x[b,t], m, t]
  H[b,m,t]  = prod_{j<=t} eps_sel[b,m,j]        (log-space cumsum on device)
  r_picked[b,t] = sum_m H[b,m,t]
  r_sum[b,t]    = sum_m (eps0+eps1)[m,t] * H[b,m,t-1]   (H[.,.,-1] = 1)
  r_other = r_sum - r_picked
  term[b,t] = unmasked_other * (rp - mx - 0.5*log1p(exp(2*(mn-mx))))
  out[b] = sum_t term[b,t]

Device layout: t on partitions (2 chunks of 128), free = (b-major, m-minor).
  T1 = x*lr (DVE), S = Tri @ (T1 + l0_bcast) via PE psum accumulation
  H = exp(S) (ACT), S2 = S + lnw_shift_bcast (DVE), WH = exp(S2) (ACT)
  r_picked/r_sum_next = segmented reduce over m (DVE)
  shift/mask/logsumexp tail: small matmuls + DVE/ACT ops.

Sharding: data-parallel over batch, 128 rows per core, 8 cores.
"""
import os
import sys

import numpy as np

for _p in ("/opt/trn_rl_repo", os.path.expanduser("~/.axon_site/_ro/trn_rl_repo")):
    if os.path.isdir(_p) and _p not in sys.path:
        sys.path.insert(0, _p)
        break

import concourse.bass as bass
import concourse.bacc as bacc
import concourse.mybir as mybir
from concourse.tile import TileContext

B, L, M = 1024, 256, 128
NCORES = 8
BLOC = B // NCORES          # 128 batch rows per core
HALF = L // 2
NB = 4                      # batch rows per free-block
FB = NB * M                 # 512 free elements per matmul/psum tile
NBLK = BLOC // NB           # 32 blocks
NCHUNK = 2                  # t-chunks of 128 partitions

F32 = mybir.dt.float32
F32R = mybir.dt.float32r
AF = mybir.ActivationFunctionType
ALU = mybir.AluOpType

USE_F32R = True             # f32r: 1 cyc/row matmul vs fp32 4 cyc/row


def _r(ap):
    return ap.bitcast(F32R) if USE_F32R else ap


def build_nc():
    nc = bacc.Bacc("TRN2", target_bir_lowering=False)
    # all fp32 constants packed into one tensor (single DMA -> single wait sem),
    # f32r matmul operands packed into a second one
    meg = nc.dram_tensor("meg", (128, 1282), F32, kind="ExternalInput")
    megr = nc.dram_tensor("megr", (128, 512), F32R, kind="ExternalInput")
    one0 = nc.dram_tensor("one0", (1, BLOC), F32, kind="ExternalInput")
    cst0 = nc.dram_tensor("cst0", (1, BLOC), F32, kind="ExternalInput")
    y = nc.dram_tensor("y", (1, BLOC), F32, kind="ExternalOutput")

    with TileContext(nc) as tc:
        with (
            tc.tile_pool(name="const", bufs=1) as cpool,
            tc.tile_pool(name="t1p", bufs=3) as t1pool,
            tc.tile_pool(name="hp", bufs=2) as hpool,
            tc.tile_pool(name="whp", bufs=2) as whpool,
            tc.tile_pool(name="small", bufs=1) as spool,
            tc.tile_pool(name="ps", bufs=2, space="PSUM") as pspool,
            tc.tile_pool(name="psm", bufs=1, space="PSUM") as psmisc,
        ):
            # ------- constants into SBUF (2 packed DMAs) -------
            MEG = cpool.tile([128, 1282], F32, tag="MEG")
            MEGR = cpool.tile([128, 512], F32R, tag="MEGR")
            nc.sync.dma_start(MEG[:], meg[:])
            nc.sync.dma_start(MEGR[:], megr[:])
            X = MEG[:, 0:256].rearrange("p (c b) -> p c b", c=NCHUNK)
            LR = MEG[:, 256:512].rearrange("p (c m) -> p c m", c=NCHUNK)
            LW = MEG[:, 512:768].rearrange("p (c m) -> p c m", c=NCHUNK)
            ONESM = MEG[:, 768:896]
            STRI = MEG[:, 896:1024]
            SHM = MEG[:, 1024:1152]
            SH2 = MEG[:, 1152:1280]
            TV = MEG[:, 1280:1282]
            L0 = MEGR[:, 0:256].rearrange("p (c m) -> p c m", c=NCHUNK)
            TRI = MEGR[:, 256:384]
            ONESR = MEGR[:, 384:512]
            ONE0 = cpool.tile([1, BLOC], F32, tag="ONE0")
            CST0 = cpool.tile([1, BLOC], F32, tag="CST0")
            nc.sync.dma_start(ONE0[:], one0[:])
            nc.sync.dma_start(CST0[:], cst0[:])

            # persistent accumulators for the reduce outputs
            RP = spool.tile([128, NCHUNK, BLOC], F32, tag="RP")
            RSN = spool.tile([128, NCHUNK, BLOC], F32, tag="RSN")

            # ------- main blocked pipeline -------
            # quarter-granularity H/WH tiles; m-axis folded 128->32 by
            # accumulating SBUF->SBUF DMAs (SWDGE), then short DVE reduces
            QB = 32
            NQ = BLOC // QB
            for q in range(NQ):
                hq = [hpool.tile([128, QB, M], F32, tag=f"H_{c}",
                                 name=f"hq_{q}_{c}")
                      for c in range(NCHUNK)]
                wq = [whpool.tile([128, QB, M], F32, tag=f"WH_{c}",
                                  name=f"wq_{q}_{c}")
                      for c in range(NCHUNK)]
                for j in range(QB // NB):
                    fb = q * (QB // NB) + j
                    bsl = slice(fb * NB, (fb + 1) * NB)
                    osl = slice(j * NB, (j + 1) * NB)
                    t1 = []
                    for c in range(NCHUNK):
                        t = t1pool.tile([128, NB, M], F32R, tag=f"T1_{c}")
                        xbc = X[:, c, bsl].unsqueeze(2).broadcast_to([128, NB, M])
                        lrbc = LR[:, c, :].unsqueeze(1).broadcast_to([128, NB, M])
                        nc.vector.tensor_tensor(t[:], xbc, lrbc, ALU.mult)
                        t1.append(t)
                    l0bc = [
                        L0[:, c, :].unsqueeze(1).broadcast_to([128, NB, M])
                        for c in range(NCHUNK)
                    ]
                    for c in range(NCHUNK):
                        sp_ = pspool.tile([128, FB], F32, tag=f"S_{c}")
                        spv = sp_[:].rearrange("p (a b) -> p a b", b=M)
                        if c == 0:
                            nc.tensor.matmul(spv, TRI, t1[0][:],
                                             start=True, stop=False)
                            nc.tensor.matmul(spv, TRI, l0bc[0],
                                             start=False, stop=True)
                        else:
                            nc.tensor.matmul(spv, TRI, t1[1][:],
                                             start=True, stop=False)
                            nc.tensor.matmul(spv, TRI, l0bc[1],
                                             start=False, stop=False)
                            nc.tensor.matmul(spv, ONESR, t1[0][:],
                                             start=False, stop=False)
                            nc.tensor.matmul(spv, ONESR, l0bc[0],
                                             start=False, stop=True)
                        nc.scalar.activation(hq[c][:, osl, :], spv, AF.Exp)
                        wbc = LW[:, c, :].unsqueeze(1).broadcast_to([128, NB, M])
                        eng = nc.vector if j % 2 == 0 else nc.gpsimd
                        eng.tensor_tensor(wq[c][:, osl, :], hq[c][:, osl, :],
                                          wbc, ALU.mult)
                qsl = slice(q * QB, (q + 1) * QB)
                for c in range(NCHUNK):
                    for hw in (hq[c], wq[c]):
                        for K in (64, 32):
                            nc.gpsimd.dma_start(hw[:, :, 0:K],
                                                hw[:, :, K:2 * K],
                                                accum_op=ALU.add)
                    nc.vector.tensor_reduce(RP[:, c, qsl], hq[c][:, :, 0:32],
                                            mybir.AxisListType.X, ALU.add)
                    nc.vector.tensor_reduce(RSN[:, c, qsl], wq[c][:, :, 0:32],
                                            mybir.AxisListType.X, ALU.add)

            # ------- tail -------
            # exclusive spin-up counts c1[t,b] via strict-lower-tri matmuls
            C1p = psmisc.tile([128, NCHUNK, BLOC], F32, tag="C1")
            nc.tensor.matmul(C1p[:, 0, :], STRI, X[:, 0, :],
                             start=True, stop=True)
            nc.tensor.matmul(C1p[:, 1, :], STRI, X[:, 1, :],
                             start=True, stop=False)
            nc.tensor.matmul(C1p[:, 1, :], ONESM, X[:, 0, :],
                             start=False, stop=True)
            # r_sum aligned: RSA[t] = RSN[t-1], RSA[0] = S0 const
            RSAp = psmisc.tile([128, NCHUNK, BLOC], F32, tag="RSA")
            nc.tensor.matmul(RSAp[:, 0, :], SHM, RSN[:, 0, :],
                             start=True, stop=False)
            nc.tensor.matmul(RSAp[:, 0, :], ONE0[:], CST0[:],
                             start=False, stop=True)
            nc.tensor.matmul(RSAp[:, 1, :], SHM, RSN[:, 1, :],
                             start=True, stop=False)
            nc.tensor.matmul(RSAp[:, 1, :], SH2, RSN[:, 0, :],
                             start=False, stop=True)
            # n_other = c1 + x*(t - 2*c1); notmask = n_other < HALF
            NM = spool.tile([128, NCHUNK, BLOC], F32, tag="NM")
            UT = spool.tile([128, NCHUNK, BLOC], F32, tag="UT")
            for c in range(NCHUNK):
                nc.vector.tensor_scalar(UT[:, c, :], C1p[:, c, :], -2.0,
                                        TV[:, c:c + 1], ALU.mult, ALU.add)
                nc.vector.tensor_tensor(UT[:, c, :], UT[:, c, :], X[:, c, :],
                                        ALU.mult)
                nc.vector.tensor_tensor(UT[:, c, :], UT[:, c, :], C1p[:, c, :],
                                        ALU.add)
                nc.vector.tensor_single_scalar(NM[:, c, :], UT[:, c, :],
                                               float(HALF) - 0.5, ALU.is_lt)
            # term = notmask * (rp - mx - 0.5*softplus(2*(mn-mx)))
            RO = spool.tile([128, NCHUNK, BLOC], F32, tag="RO")
            MX = spool.tile([128, NCHUNK, BLOC], F32, tag="MX")
            MN = spool.tile([128, NCHUNK, BLOC], F32, tag="MN")
            SPt = spool.tile([128, NCHUNK, BLOC], F32, tag="SPt")
            TERM = spool.tile([128, NCHUNK, BLOC], F32, tag="TERM")
            nc.vector.tensor_tensor(RO[:], RSAp[:], RP[:], ALU.subtract)
            nc.vector.tensor_tensor(MX[:], RP[:], RO[:], ALU.max)
            nc.vector.tensor_tensor(MN[:], RP[:], RO[:], ALU.min)
            nc.vector.tensor_tensor(MN[:], MN[:], MX[:], ALU.subtract)
            # softplus(2*(mn-mx)) = ln(1 + exp(2*(mn-mx))) via Exp then Ln(x+1)
            nc.scalar.activation(SPt[:], MN[:], AF.Exp, scale=2.0)
            nc.scalar.activation(SPt[:], SPt[:], AF.Ln, bias=1.0)
            nc.vector.tensor_tensor(MX[:], RP[:], MX[:], ALU.subtract)
            nc.vector.scalar_tensor_tensor(TERM[:], SPt[:], -0.5, MX[:],
                                           ALU.mult, ALU.add)
            nc.vector.tensor_tensor(TERM[:], TERM[:], NM[:], ALU.mult)
            # out[b] = sum_t term
            YPp = psmisc.tile([1, NCHUNK * BLOC], F32, tag="YP")
            nc.tensor.matmul(YPp[:], ONESM[:, 0:1],
                             TERM[:].rearrange("p a b -> p (a b)"),
                             start=True, stop=True)
            YS = spool.tile([1, NCHUNK * BLOC], F32, tag="YS")
            nc.scalar.activation(YS[:], YPp[:], AF.Copy)
            YF = spool.tile([1, BLOC], F32, tag="YF")
            nc.vector.tensor_tensor(YF[:], YS[0:1, 0:BLOC],
                                    YS[0:1, BLOC:2 * BLOC], ALU.add)
            nc.sync.dma_start(y[:], YF[:])
    nc.compile()
    return nc


def host_tables(inputs, epsilon):
    x = np.asarray(inputs).astype(np.float32)        # (B, L)
    eps = np.asarray(epsilon).astype(np.float32)     # (2, M, L)
    eps0, eps1 = eps[0], eps[1]
    le0 = np.log(eps0)                               # (M, L)
    le1 = np.log(eps1)
    w = eps0 + eps1
    lnw_sh = np.zeros((M, L), np.float32)   # now the *linear* shifted weight table
    lnw_sh[:, : L - 1] = w[:, 1:]
    s0 = np.float32(w[:, 0].sum(dtype=np.float64))

    ar = np.arange(128)
    tri = np.asarray(ar[:, None] <= ar[None, :], np.float32)
    stri = np.asarray(ar[:, None] < ar[None, :], np.float32)
    onesm = np.ones((128, 128), np.float32)
    shm = np.asarray(ar[:, None] == (ar[None, :] - 1), np.float32)
    sh2 = np.asarray((ar[:, None] == 127) & (ar[None, :] == 0), np.float32)
    tv = (ar[:, None] + 128.0 * np.arange(NCHUNK)[None, :]).astype(np.float32)

    def chunked(a_t):  # (L, K) -> (128, 2*K) with [:, c*K:(c+1)*K] = chunk c
        return np.concatenate([a_t[c * 128:(c + 1) * 128] for c in range(NCHUNK)],
                              axis=1)

    lr_t = np.ascontiguousarray((le1 - le0).T)       # (L, M)
    l0_t = np.ascontiguousarray(le0.T)
    lnw_t = np.ascontiguousarray(lnw_sh.T)
    xt_all = np.ascontiguousarray(x.T)               # (L, B)

    meg_fixed = [chunked(lr_t), chunked(lnw_t), onesm, stri, shm, sh2, tv]
    megr = np.ascontiguousarray(
        np.concatenate([chunked(l0_t), tri, onesm], axis=1))
    tables = {
        "megr": megr,
        "one0": np.asarray(np.arange(BLOC)[None, :] == 0, np.float32),
        "cst0": np.full((1, BLOC), s0, np.float32),
    }
    return tables, meg_fixed, xt_all, chunked


_NC_CACHE = {}


def get_nc():
    if "nc" not in _NC_CACHE:
        _NC_CACHE["nc"] = build_nc()
    return _NC_CACHE["nc"]


def kernel(inputs, epsilon):
    from concourse.bass_utils import run_bass_kernel_spmd

    tables, meg_fixed, xt_all, chunked = host_tables(inputs, epsilon)
    nc = get_nc()
    in_maps = []
    for k in range(NCORES):
        m = dict(tables)
        xt_core = np.ascontiguousarray(xt_all[:, k * BLOC:(k + 1) * BLOC])
        m["meg"] = np.ascontiguousarray(
            np.concatenate([chunked(xt_core)] + meg_fixed, axis=1))
        in_maps.append(m)
    res = run_bass_kernel_spmd(nc, in_maps, core_ids=list(range(NCORES)))
    out = np.empty((B,), np.float32)
    for k in range(NCORES):
        out[k * BLOC:(k + 1) * BLOC] = np.asarray(res.results[k]["y"]).reshape(-1)
    return out


# revision 10
# speedup vs baseline: 2.5120x; 2.5120x over previous
"""
Trainium2 Bass kernel for nn_ARqGPS (autoregressive qGPS log-amplitude).

Math (validated vs reference):
  eps_sel[b,m,t] = epsilon[x[b,t], m, t]
  H[b,m,t]  = prod_{j<=t} eps_sel[b,m,j]        (log-space cumsum on device)
  r_picked[b,t] = sum_m H[b,m,t]
  r_sum[b,t]    = sum_m (eps0+eps1)[m,t] * H[b,m,t-1]   (H[.,.,-1] = 1)
  r_other = r_sum - r_picked
  term[b,t] = unmasked_other * (rp - mx - 0.5*log1p(exp(2*(mn-mx))))
  out[b] = sum_t term[b,t]

Device layout: t on partitions (2 chunks of 128), free = (b-major, m-minor).
  T1 = x*lr (DVE), S = Tri @ (T1 + l0_bcast) via PE psum accumulation
  H = exp(S) (ACT), S2 = S + lnw_shift_bcast (DVE), WH = exp(S2) (ACT)
  r_picked/r_sum_next = segmented reduce over m (DVE)
  shift/mask/logsumexp tail: small matmuls + DVE/ACT ops.

Sharding: data-parallel over batch, 128 rows per core, 8 cores.
"""
import os
import sys

import numpy as np

for _p in ("/opt/trn_rl_repo", os.path.expanduser("~/.axon_site/_ro/trn_rl_repo")):
    if os.path.isdir(_p) and _p not in sys.path:
        sys.path.insert(0, _p)
        break

import concourse.bass as bass
import concourse.bacc as bacc
import concourse.mybir as mybir
from concourse.tile import TileContext

B, L, M = 1024, 256, 128
NCORES = 8
BLOC = B // NCORES          # 128 batch rows per core
HALF = L // 2
NB = 4                      # batch rows per free-block
FB = NB * M                 # 512 free elements per matmul/psum tile
NBLK = BLOC // NB           # 32 blocks
NCHUNK = 2                  # t-chunks of 128 partitions

F32 = mybir.dt.float32
F32R = mybir.dt.float32r
AF = mybir.ActivationFunctionType
ALU = mybir.AluOpType

USE_F32R = True             # f32r: 1 cyc/row matmul vs fp32 4 cyc/row


def _r(ap):
    return ap.bitcast(F32R) if USE_F32R else ap


def build_nc():
    nc = bacc.Bacc("TRN2", target_bir_lowering=False)
    # all fp32 constants packed into one tensor (single DMA -> single wait sem),
    # f32r matmul operands packed into a second one
    meg = nc.dram_tensor("meg", (128, 1282), F32, kind="ExternalInput")
    megr = nc.dram_tensor("megr", (128, 512), F32R, kind="ExternalInput")
    one0 = nc.dram_tensor("one0", (1, BLOC), F32, kind="ExternalInput")
    cst0 = nc.dram_tensor("cst0", (1, BLOC), F32, kind="ExternalInput")
    y = nc.dram_tensor("y", (1, BLOC), F32, kind="ExternalOutput")

    with TileContext(nc) as tc:
        with (
            tc.tile_pool(name="const", bufs=1) as cpool,
            tc.tile_pool(name="t1p", bufs=3) as t1pool,
            tc.tile_pool(name="hp", bufs=2) as hpool,
            tc.tile_pool(name="whp", bufs=2) as whpool,
            tc.tile_pool(name="small", bufs=1) as spool,
            tc.tile_pool(name="ps", bufs=2, space="PSUM") as pspool,
            tc.tile_pool(name="psm", bufs=1, space="PSUM") as psmisc,
        ):
            # ------- constants into SBUF (2 packed DMAs) -------
            MEG = cpool.tile([128, 1282], F32, tag="MEG")
            MEGR = cpool.tile([128, 512], F32R, tag="MEGR")
            nc.sync.dma_start(MEG[:], meg[:])
            nc.sync.dma_start(MEGR[:], megr[:])
            X = MEG[:, 0:256].rearrange("p (c b) -> p c b", c=NCHUNK)
            LR = MEG[:, 256:512].rearrange("p (c m) -> p c m", c=NCHUNK)
            LW = MEG[:, 512:768].rearrange("p (c m) -> p c m", c=NCHUNK)
            ONESM = MEG[:, 768:896]
            STRI = MEG[:, 896:1024]
            SHM = MEG[:, 1024:1152]
            SH2 = MEG[:, 1152:1280]
            TV = MEG[:, 1280:1282]
            L0 = MEGR[:, 0:256].rearrange("p (c m) -> p c m", c=NCHUNK)
            TRI = MEGR[:, 256:384]
            ONESR = MEGR[:, 384:512]
            ONE0 = cpool.tile([1, BLOC], F32, tag="ONE0")
            CST0 = cpool.tile([1, BLOC], F32, tag="CST0")
            nc.sync.dma_start(ONE0[:], one0[:])
            nc.sync.dma_start(CST0[:], cst0[:])

            # persistent accumulators for the reduce outputs
            RP = spool.tile([128, NCHUNK, BLOC], F32, tag="RP")
            RSN = spool.tile([128, NCHUNK, BLOC], F32, tag="RSN")

            # ------- main blocked pipeline -------
            # quarter-granularity H/WH tiles; m-axis folded 128->32 by
            # accumulating SBUF->SBUF DMAs (SWDGE), then short DVE reduces
            QB = 32
            NQ = BLOC // QB
            for q in range(NQ):
                hq = [hpool.tile([128, QB, M], F32, tag=f"H_{c}",
                                 name=f"hq_{q}_{c}")
                      for c in range(NCHUNK)]
                wq = [whpool.tile([128, QB, M], F32, tag=f"WH_{c}",
                                  name=f"wq_{q}_{c}")
                      for c in range(NCHUNK)]
                for j in range(QB // NB):
                    fb = q * (QB // NB) + j
                    bsl = slice(fb * NB, (fb + 1) * NB)
                    osl = slice(j * NB, (j + 1) * NB)
                    t1 = []
                    for c in range(NCHUNK):
                        t = t1pool.tile([128, NB, M], F32R, tag=f"T1_{c}")
                        xbc = X[:, c, bsl].unsqueeze(2).broadcast_to([128, NB, M])
                        lrbc = LR[:, c, :].unsqueeze(1).broadcast_to([128, NB, M])
                        nc.vector.tensor_tensor(t[:], xbc, lrbc, ALU.mult)
                        t1.append(t)
                    l0bc = [
                        L0[:, c, :].unsqueeze(1).broadcast_to([128, NB, M])
                        for c in range(NCHUNK)
                    ]
                    for c in range(NCHUNK):
                        sp_ = pspool.tile([128, FB], F32, tag=f"S_{c}")
                        spv = sp_[:].rearrange("p (a b) -> p a b", b=M)
                        if c == 0:
                            nc.tensor.matmul(spv, TRI, t1[0][:],
                                             start=True, stop=False)
                            nc.tensor.matmul(spv, TRI, l0bc[0],
                                             start=False, stop=True)
                        else:
                            nc.tensor.matmul(spv, TRI, t1[1][:],
                                             start=True, stop=False)
                            nc.tensor.matmul(spv, TRI, l0bc[1],
                                             start=False, stop=False)
                            nc.tensor.matmul(spv, ONESR, t1[0][:],
                                             start=False, stop=False)
                            nc.tensor.matmul(spv, ONESR, l0bc[0],
                                             start=False, stop=True)
                        nc.scalar.activation(hq[c][:, osl, :], spv, AF.Exp)
                        wbc = LW[:, c, :].unsqueeze(1).broadcast_to([128, NB, M])
                        nc.gpsimd.tensor_tensor(wq[c][:, osl, :], hq[c][:, osl, :],
                                          wbc, ALU.mult)
                qsl = slice(q * QB, (q + 1) * QB)
                for c in range(NCHUNK):
                    nc.vector.tensor_reduce(RP[:, c, qsl], hq[c][:],
                                            mybir.AxisListType.X, ALU.add)
                    nc.vector.tensor_reduce(RSN[:, c, qsl], wq[c][:],
                                            mybir.AxisListType.X, ALU.add)

            # ------- tail -------
            # exclusive spin-up counts c1[t,b] via strict-lower-tri matmuls
            C1p = psmisc.tile([128, NCHUNK, BLOC], F32, tag="C1")
            nc.tensor.matmul(C1p[:, 0, :], STRI, X[:, 0, :],
                             start=True, stop=True)
            nc.tensor.matmul(C1p[:, 1, :], STRI, X[:, 1, :],
                             start=True, stop=False)
            nc.tensor.matmul(C1p[:, 1, :], ONESM, X[:, 0, :],
                             start=False, stop=True)
            # r_sum aligned: RSA[t] = RSN[t-1], RSA[0] = S0 const
            RSAp = psmisc.tile([128, NCHUNK, BLOC], F32, tag="RSA")
            nc.tensor.matmul(RSAp[:, 0, :], SHM, RSN[:, 0, :],
                             start=True, stop=False)
            nc.tensor.matmul(RSAp[:, 0, :], ONE0[:], CST0[:],
                             start=False, stop=True)
            nc.tensor.matmul(RSAp[:, 1, :], SHM, RSN[:, 1, :],
                             start=True, stop=False)
            nc.tensor.matmul(RSAp[:, 1, :], SH2, RSN[:, 0, :],
                             start=False, stop=True)
            # n_other = c1 + x*(t - 2*c1); notmask = n_other < HALF
            NM = spool.tile([128, NCHUNK, BLOC], F32, tag="NM")
            UT = spool.tile([128, NCHUNK, BLOC], F32, tag="UT")
            for c in range(NCHUNK):
                nc.vector.tensor_scalar(UT[:, c, :], C1p[:, c, :], -2.0,
                                        TV[:, c:c + 1], ALU.mult, ALU.add)
                nc.vector.tensor_tensor(UT[:, c, :], UT[:, c, :], X[:, c, :],
                                        ALU.mult)
                nc.vector.tensor_tensor(UT[:, c, :], UT[:, c, :], C1p[:, c, :],
                                        ALU.add)
                nc.vector.tensor_single_scalar(NM[:, c, :], UT[:, c, :],
                                               float(HALF) - 0.5, ALU.is_lt)
            # term = notmask * (rp - mx - 0.5*softplus(2*(mn-mx)))
            RO = spool.tile([128, NCHUNK, BLOC], F32, tag="RO")
            MX = spool.tile([128, NCHUNK, BLOC], F32, tag="MX")
            MN = spool.tile([128, NCHUNK, BLOC], F32, tag="MN")
            SPt = spool.tile([128, NCHUNK, BLOC], F32, tag="SPt")
            TERM = spool.tile([128, NCHUNK, BLOC], F32, tag="TERM")
            nc.vector.tensor_tensor(RO[:], RSAp[:], RP[:], ALU.subtract)
            nc.vector.tensor_tensor(MX[:], RP[:], RO[:], ALU.max)
            nc.vector.tensor_tensor(MN[:], RP[:], RO[:], ALU.min)
            nc.vector.tensor_tensor(MN[:], MN[:], MX[:], ALU.subtract)
            # softplus(2*(mn-mx)) = ln(1 + exp(2*(mn-mx))) via Exp then Ln(x+1)
            nc.scalar.activation(SPt[:], MN[:], AF.Exp, scale=2.0)
            nc.scalar.activation(SPt[:], SPt[:], AF.Ln, bias=1.0)
            nc.vector.tensor_tensor(MX[:], RP[:], MX[:], ALU.subtract)
            nc.vector.scalar_tensor_tensor(TERM[:], SPt[:], -0.5, MX[:],
                                           ALU.mult, ALU.add)
            nc.vector.tensor_tensor(TERM[:], TERM[:], NM[:], ALU.mult)
            # out[b] = sum_t term
            YPp = psmisc.tile([1, NCHUNK * BLOC], F32, tag="YP")
            nc.tensor.matmul(YPp[:], ONESM[:, 0:1],
                             TERM[:].rearrange("p a b -> p (a b)"),
                             start=True, stop=True)
            YS = spool.tile([1, NCHUNK * BLOC], F32, tag="YS")
            nc.scalar.activation(YS[:], YPp[:], AF.Copy)
            YF = spool.tile([1, BLOC], F32, tag="YF")
            nc.vector.tensor_tensor(YF[:], YS[0:1, 0:BLOC],
                                    YS[0:1, BLOC:2 * BLOC], ALU.add)
            nc.sync.dma_start(y[:], YF[:])
    nc.compile()
    return nc


def host_tables(inputs, epsilon):
    x = np.asarray(inputs).astype(np.float32)        # (B, L)
    eps = np.asarray(epsilon).astype(np.float32)     # (2, M, L)
    eps0, eps1 = eps[0], eps[1]
    le0 = np.log(eps0)                               # (M, L)
    le1 = np.log(eps1)
    w = eps0 + eps1
    lnw_sh = np.zeros((M, L), np.float32)   # now the *linear* shifted weight table
    lnw_sh[:, : L - 1] = w[:, 1:]
    s0 = np.float32(w[:, 0].sum(dtype=np.float64))

    ar = np.arange(128)
    tri = np.asarray(ar[:, None] <= ar[None, :], np.float32)
    stri = np.asarray(ar[:, None] < ar[None, :], np.float32)
    onesm = np.ones((128, 128), np.float32)
    shm = np.asarray(ar[:, None] == (ar[None, :] - 1), np.float32)
    sh2 = np.asarray((ar[:, None] == 127) & (ar[None, :] == 0), np.float32)
    tv = (ar[:, None] + 128.0 * np.arange(NCHUNK)[None, :]).astype(np.float32)

    def chunked(a_t):  # (L, K) -> (128, 2*K) with [:, c*K:(c+1)*K] = chunk c
        return np.concatenate([a_t[c * 128:(c + 1) * 128] for c in range(NCHUNK)],
                              axis=1)

    lr_t = np.ascontiguousarray((le1 - le0).T)       # (L, M)
    l0_t = np.ascontiguousarray(le0.T)
    lnw_t = np.ascontiguousarray(lnw_sh.T)
    xt_all = np.ascontiguousarray(x.T)               # (L, B)

    meg_fixed = [chunked(lr_t), chunked(lnw_t), onesm, stri, shm, sh2, tv]
    megr = np.ascontiguousarray(
        np.concatenate([chunked(l0_t), tri, onesm], axis=1))
    tables = {
        "megr": megr,
        "one0": np.asarray(np.arange(BLOC)[None, :] == 0, np.float32),
        "cst0": np.full((1, BLOC), s0, np.float32),
    }
    return tables, meg_fixed, xt_all, chunked


_NC_CACHE = {}


def get_nc():
    if "nc" not in _NC_CACHE:
        _NC_CACHE["nc"] = build_nc()
    return _NC_CACHE["nc"]


def kernel(inputs, epsilon):
    from concourse.bass_utils import run_bass_kernel_spmd

    tables, meg_fixed, xt_all, chunked = host_tables(inputs, epsilon)
    nc = get_nc()
    in_maps = []
    for k in range(NCORES):
        m = dict(tables)
        xt_core = np.ascontiguousarray(xt_all[:, k * BLOC:(k + 1) * BLOC])
        m["meg"] = np.ascontiguousarray(
            np.concatenate([chunked(xt_core)] + meg_fixed, axis=1))
        in_maps.append(m)
    res = run_bass_kernel_spmd(nc, in_maps, core_ids=list(range(NCORES)))
    out = np.empty((B,), np.float32)
    for k in range(NCORES):
        out[k * BLOC:(k + 1) * BLOC] = np.asarray(res.results[k]["y"]).reshape(-1)
    return out


# revision 11
# speedup vs baseline: 3.3387x; 1.3291x over previous
"""
Trainium2 Bass kernel for nn_ARqGPS (autoregressive qGPS log-amplitude).

Math (validated vs reference):
  eps_sel[b,m,t] = epsilon[x[b,t], m, t]
  H[b,m,t]  = prod_{j<=t} eps_sel[b,m,j]        (log-space cumsum on device)
  r_picked[b,t] = sum_m H[b,m,t]
  r_sum[b,t]    = sum_m (eps0+eps1)[m,t] * H[b,m,t-1]   (H[.,.,-1] = 1)
  r_other = r_sum - r_picked
  term[b,t] = unmasked_other * (rp - mx - 0.5*log1p(exp(2*(mn-mx))))
  out[b] = sum_t term[b,t]

Device layout: t on partitions (2 chunks of 128), free = (b-major, m-minor).
  T1 = x*lr (DVE), S = Tri @ (T1 + l0_bcast) via PE psum accumulation
  H = exp(S) (ACT), S2 = S + lnw_shift_bcast (DVE), WH = exp(S2) (ACT)
  r_picked/r_sum_next = segmented reduce over m (DVE)
  shift/mask/logsumexp tail: small matmuls + DVE/ACT ops.

Sharding: data-parallel over batch, 128 rows per core, 8 cores.
"""
import os
import sys

import numpy as np

for _p in ("/opt/trn_rl_repo", os.path.expanduser("~/.axon_site/_ro/trn_rl_repo")):
    if os.path.isdir(_p) and _p not in sys.path:
        sys.path.insert(0, _p)
        break

import concourse.bass as bass
import concourse.bacc as bacc
import concourse.mybir as mybir
from concourse.tile import TileContext

B, L, M = 1024, 256, 128
NCORES = 8
BLOC = B // NCORES          # 128 batch rows per core
HALF = L // 2
NB = 4                      # batch rows per free-block
FB = NB * M                 # 512 free elements per matmul/psum tile
NBLK = BLOC // NB           # 32 blocks
NCHUNK = 2                  # t-chunks of 128 partitions

F32 = mybir.dt.float32
F32R = mybir.dt.float32r
AF = mybir.ActivationFunctionType
ALU = mybir.AluOpType

USE_F32R = True             # f32r: 1 cyc/row matmul vs fp32 4 cyc/row


def _r(ap):
    return ap.bitcast(F32R) if USE_F32R else ap


def build_nc():
    nc = bacc.Bacc("TRN2", target_bir_lowering=False)
    # all fp32 constants packed into one tensor (single DMA -> single wait sem),
    # f32r matmul operands packed into a second one
    meg = nc.dram_tensor("meg", (128, 1282), F32, kind="ExternalInput")
    megr = nc.dram_tensor("megr", (128, 512), F32R, kind="ExternalInput")
    one0 = nc.dram_tensor("one0", (1, BLOC), F32, kind="ExternalInput")
    cst0 = nc.dram_tensor("cst0", (1, BLOC), F32, kind="ExternalInput")
    y = nc.dram_tensor("y", (1, BLOC), F32, kind="ExternalOutput")

    with TileContext(nc) as tc:
        with (
            tc.tile_pool(name="const", bufs=1) as cpool,
            tc.tile_pool(name="t1p", bufs=3) as t1pool,
            tc.tile_pool(name="hp", bufs=2) as hpool,
            tc.tile_pool(name="whp", bufs=2) as whpool,
            tc.tile_pool(name="small", bufs=1) as spool,
            tc.tile_pool(name="ps", bufs=2, space="PSUM") as pspool,
            tc.tile_pool(name="psm", bufs=1, space="PSUM") as psmisc,
        ):
            # ------- constants into SBUF (2 packed DMAs) -------
            MEG = cpool.tile([128, 1282], F32, tag="MEG")
            MEGR = cpool.tile([128, 512], F32R, tag="MEGR")
            nc.sync.dma_start(MEG[:], meg[:])
            nc.sync.dma_start(MEGR[:], megr[:])
            X = MEG[:, 0:256].rearrange("p (c b) -> p c b", c=NCHUNK)
            LR = MEG[:, 256:512].rearrange("p (c m) -> p c m", c=NCHUNK)
            LW = MEG[:, 512:768].rearrange("p (c m) -> p c m", c=NCHUNK)
            ONESM = MEG[:, 768:896]
            STRI = MEG[:, 896:1024]
            SHM = MEG[:, 1024:1152]
            SH2 = MEG[:, 1152:1280]
            TV = MEG[:, 1280:1282]
            L0 = MEGR[:, 0:256].rearrange("p (c m) -> p c m", c=NCHUNK)
            TRI = MEGR[:, 256:384]
            ONESR = MEGR[:, 384:512]
            ONE0 = cpool.tile([1, BLOC], F32, tag="ONE0")
            CST0 = cpool.tile([1, BLOC], F32, tag="CST0")
            nc.sync.dma_start(ONE0[:], one0[:])
            nc.sync.dma_start(CST0[:], cst0[:])

            # persistent accumulators for the reduce outputs
            RP = spool.tile([128, NCHUNK, BLOC], F32, tag="RP")
            RSN = spool.tile([128, NCHUNK, BLOC], F32, tag="RSN")

            # ------- main blocked pipeline -------
            for fb in range(NBLK):
                bsl = slice(fb * NB, (fb + 1) * NB)
                # ~1/3 of the T1 mults go to GpSimd to balance DVE
                t1eng = nc.gpsimd if fb % 8 < 3 else nc.vector
                t1 = []
                for c in range(NCHUNK):
                    t = t1pool.tile([128, NB, M], F32R, tag=f"T1_{c}")
                    xbc = X[:, c, bsl].unsqueeze(2).broadcast_to([128, NB, M])
                    lrbc = LR[:, c, :].unsqueeze(1).broadcast_to([128, NB, M])
                    t1eng.tensor_tensor(t[:], xbc, lrbc, ALU.mult)
                    t1.append(t)
                l0bc = [
                    L0[:, c, :].unsqueeze(1).broadcast_to([128, NB, M])
                    for c in range(NCHUNK)
                ]
                for c in range(NCHUNK):
                    sp_ = pspool.tile([128, FB], F32, tag=f"S_{c}")
                    spv = sp_[:].rearrange("p (a b) -> p a b", b=M)
                    if c == 0:
                        nc.tensor.matmul(spv, TRI, t1[0][:],
                                         start=True, stop=False)
                        nc.tensor.matmul(spv, TRI, l0bc[0],
                                         start=False, stop=True)
                    else:
                        nc.tensor.matmul(spv, TRI, t1[1][:],
                                         start=True, stop=False)
                        nc.tensor.matmul(spv, TRI, l0bc[1],
                                         start=False, stop=False)
                        nc.tensor.matmul(spv, ONESR, t1[0][:],
                                         start=False, stop=False)
                        nc.tensor.matmul(spv, ONESR, l0bc[0],
                                         start=False, stop=True)
                    ht = hpool.tile([128, NB, M], F32, tag=f"H_{c}")
                    nc.scalar.activation(ht[:], spv, AF.Exp)
                    wh = whpool.tile([128, NB, M], F32, tag=f"WH_{c}")
                    wbc = LW[:, c, :].unsqueeze(1).broadcast_to([128, NB, M])
                    nc.gpsimd.tensor_tensor(wh[:], ht[:], wbc, ALU.mult)
                    nc.vector.tensor_reduce(RP[:, c, bsl], ht[:],
                                            mybir.AxisListType.X, ALU.add)
                    nc.vector.tensor_reduce(RSN[:, c, bsl], wh[:],
                                            mybir.AxisListType.X, ALU.add)

            # ------- tail -------
            # exclusive spin-up counts c1[t,b] via strict-lower-tri matmuls
            C1p = psmisc.tile([128, NCHUNK, BLOC], F32, tag="C1")
            nc.tensor.matmul(C1p[:, 0, :], STRI, X[:, 0, :],
                             start=True, stop=True)
            nc.tensor.matmul(C1p[:, 1, :], STRI, X[:, 1, :],
                             start=True, stop=False)
            nc.tensor.matmul(C1p[:, 1, :], ONESM, X[:, 0, :],
                             start=False, stop=True)
            # r_sum aligned: RSA[t] = RSN[t-1], RSA[0] = S0 const
            RSAp = psmisc.tile([128, NCHUNK, BLOC], F32, tag="RSA")
            nc.tensor.matmul(RSAp[:, 0, :], SHM, RSN[:, 0, :],
                             start=True, stop=False)
            nc.tensor.matmul(RSAp[:, 0, :], ONE0[:], CST0[:],
                             start=False, stop=True)
            nc.tensor.matmul(RSAp[:, 1, :], SHM, RSN[:, 1, :],
                             start=True, stop=False)
            nc.tensor.matmul(RSAp[:, 1, :], SH2, RSN[:, 0, :],
                             start=False, stop=True)
            # n_other = c1 + x*(t - 2*c1); notmask = n_other < HALF
            NM = spool.tile([128, NCHUNK, BLOC], F32, tag="NM")
            UT = spool.tile([128, NCHUNK, BLOC], F32, tag="UT")
            for c in range(NCHUNK):
                nc.vector.tensor_scalar(UT[:, c, :], C1p[:, c, :], -2.0,
                                        TV[:, c:c + 1], ALU.mult, ALU.add)
                nc.vector.tensor_tensor(UT[:, c, :], UT[:, c, :], X[:, c, :],
                                        ALU.mult)
                nc.vector.tensor_tensor(UT[:, c, :], UT[:, c, :], C1p[:, c, :],
                                        ALU.add)
                nc.vector.tensor_single_scalar(NM[:, c, :], UT[:, c, :],
                                               float(HALF) - 0.5, ALU.is_lt)
            # term = notmask * (rp - mx - 0.5*softplus(2*(mn-mx)))
            RO = spool.tile([128, NCHUNK, BLOC], F32, tag="RO")
            MX = spool.tile([128, NCHUNK, BLOC], F32, tag="MX")
            MN = spool.tile([128, NCHUNK, BLOC], F32, tag="MN")
            SPt = spool.tile([128, NCHUNK, BLOC], F32, tag="SPt")
            TERM = spool.tile([128, NCHUNK, BLOC], F32, tag="TERM")
            nc.vector.tensor_tensor(RO[:], RSAp[:], RP[:], ALU.subtract)
            nc.vector.tensor_tensor(MX[:], RP[:], RO[:], ALU.max)
            nc.vector.tensor_tensor(MN[:], RP[:], RO[:], ALU.min)
            nc.vector.tensor_tensor(MN[:], MN[:], MX[:], ALU.subtract)
            # softplus(2*(mn-mx)) = ln(1 + exp(2*(mn-mx))) via Exp then Ln(x+1)
            nc.scalar.activation(SPt[:], MN[:], AF.Exp, scale=2.0)
            nc.scalar.activation(SPt[:], SPt[:], AF.Ln, bias=1.0)
            nc.vector.tensor_tensor(MX[:], RP[:], MX[:], ALU.subtract)
            nc.vector.scalar_tensor_tensor(TERM[:], SPt[:], -0.5, MX[:],
                                           ALU.mult, ALU.add)
            nc.vector.tensor_tensor(TERM[:], TERM[:], NM[:], ALU.mult)
            # out[b] = sum_t term
            YPp = psmisc.tile([1, NCHUNK * BLOC], F32, tag="YP")
            nc.tensor.matmul(YPp[:], ONESM[:, 0:1],
                             TERM[:].rearrange("p a b -> p (a b)"),
                             start=True, stop=True)
            YS = spool.tile([1, NCHUNK * BLOC], F32, tag="YS")
            nc.scalar.activation(YS[:], YPp[:], AF.Copy)
            YF = spool.tile([1, BLOC], F32, tag="YF")
            nc.vector.tensor_tensor(YF[:], YS[0:1, 0:BLOC],
                                    YS[0:1, BLOC:2 * BLOC], ALU.add)
            nc.sync.dma_start(y[:], YF[:])
    nc.compile()
    return nc


def host_tables(inputs, epsilon):
    x = np.asarray(inputs).astype(np.float32)        # (B, L)
    eps = np.asarray(epsilon).astype(np.float32)     # (2, M, L)
    eps0, eps1 = eps[0], eps[1]
    le0 = np.log(eps0)                               # (M, L)
    le1 = np.log(eps1)
    w = eps0 + eps1
    lnw_sh = np.zeros((M, L), np.float32)   # now the *linear* shifted weight table
    lnw_sh[:, : L - 1] = w[:, 1:]
    s0 = np.float32(w[:, 0].sum(dtype=np.float64))

    ar = np.arange(128)
    tri = np.asarray(ar[:, None] <= ar[None, :], np.float32)
    stri = np.asarray(ar[:, None] < ar[None, :], np.float32)
    onesm = np.ones((128, 128), np.float32)
    shm = np.asarray(ar[:, None] == (ar[None, :] - 1), np.float32)
    sh2 = np.asarray((ar[:, None] == 127) & (ar[None, :] == 0), np.float32)
    tv = (ar[:, None] + 128.0 * np.arange(NCHUNK)[None, :]).astype(np.float32)

    def chunked(a_t):  # (L, K) -> (128, 2*K) with [:, c*K:(c+1)*K] = chunk c
        return np.concatenate([a_t[c * 128:(c + 1) * 128] for c in range(NCHUNK)],
                              axis=1)

    lr_t = np.ascontiguousarray((le1 - le0).T)       # (L, M)
    l0_t = np.ascontiguousarray(le0.T)
    lnw_t = np.ascontiguousarray(lnw_sh.T)
    xt_all = np.ascontiguousarray(x.T)               # (L, B)

    meg_fixed = [chunked(lr_t), chunked(lnw_t), onesm, stri, shm, sh2, tv]
    megr = np.ascontiguousarray(
        np.concatenate([chunked(l0_t), tri, onesm], axis=1))
    tables = {
        "megr": megr,
        "one0": np.asarray(np.arange(BLOC)[None, :] == 0, np.float32),
        "cst0": np.full((1, BLOC), s0, np.float32),
    }
    return tables, meg_fixed, xt_all, chunked


_NC_CACHE = {}


def get_nc():
    if "nc" not in _NC_CACHE:
        _NC_CACHE["nc"] = build_nc()
    return _NC_CACHE["nc"]


def kernel(inputs, epsilon):
    from concourse.bass_utils import run_bass_kernel_spmd

    tables, meg_fixed, xt_all, chunked = host_tables(inputs, epsilon)
    nc = get_nc()
    in_maps = []
    for k in range(NCORES):
        m = dict(tables)
        xt_core = np.ascontiguousarray(xt_all[:, k * BLOC:(k + 1) * BLOC])
        m["meg"] = np.ascontiguousarray(
            np.concatenate([chunked(xt_core)] + meg_fixed, axis=1))
        in_maps.append(m)
    res = run_bass_kernel_spmd(nc, in_maps, core_ids=list(range(NCORES)))
    out = np.empty((B,), np.float32)
    for k in range(NCORES):
        out[k * BLOC:(k + 1) * BLOC] = np.asarray(res.results[k]["y"]).reshape(-1)
    return out
